# revision 32
# baseline (speedup 1.0000x reference)
"""Causal multi-head decoder attention on 8 Trainium2 NeuronCores.

Problem shapes (hardcoded): x [B=2, S=2048, D=1024], 16 heads x d_head=64.
Sharding: core c -> (batch b = c//4, head-group hg = c%4 covering 4 heads).
Attention is fully head-local; each core computes the partial output
projection for its 4 heads, and the host sums the 4 partials per batch
(the "output projection all-reduce") during unshard.

On-device layout strategy (per core):
  - host provides xT = x[b].T  [1024, 2048] so Q/K projections directly
    produce qT/kT [64, S] (head-dim on partitions) with no transposes.
  - Q/K projections produce head PAIRS stacked in partition halves
    (qkT[:, r, :]: rows 0:64 = head 2r', 64:128 = head 2r'+1), so the
    score matmuls for a head pair run as two concurrent 64-row-group
    matmuls on the PE array (tile_position) with NO duplication DMAs.
  - V is computed in [S, 64] orientation (x-chunk stationary) and stored
    interleaved with a ones-column per head: vaug [128, 16sc, 4h*65].
    The ones-column makes the attn@V matmul also produce the softmax
    denominator row (zaug [65, 512] = 64 z rows + 1 denom row).
  - scoresT [ki, qi] = kT-chunk.T @ qT-tile (contraction over d_head=64).
    exp() on the scalar engine; causal mask applied multiplicatively on
    the diagonal 128-blocks only; for diagonal chunks the score matmul /
    exp / attn@V are all narrowed to the un-masked qi column range.
  - kc ascending accumulation for zaug; the kc loop is software-pipelined
    depth 2 (scores(kc+1) issued before attn@V(kc)).
  - 1/sqrt(d_head) folded into the K weights host-side.
  - softmax 1/denom via DVE reciprocal_approx_fast on the zaug denom row
    (f32, no scalar-engine work), broadcast across partitions with a
    K=1 f32r matmul against a ones row (f32r streams at full rate for
    free dims >= 256, so no f16 conversion pass is needed).
  - output projection: O stacked per head-pair so contraction is 128-wide.
    Output tiles are staged into an SBUF buffer per s-tile and shipped
    with ONE dma per s-tile on the sync queue.
  - engine load-balancing across the qt loop: the scalar-engine exp load
    grows linearly with qt (causal) while tensor work shrinks, so output
    projections are deferred into late qt windows as PE filler work:
      qt2 <- outproj(st0), qt3 <- outproj(st1) + outproj(st2),
      tail <- outproj(st3); fillers are spread EVENLY over the kc loop.
  - input DMA on sync+gpsimd queues only (scalar queue stays clear for
    the exp stream), first s-tile chunks first.
"""

import os as _os

import numpy as np

import concourse.bass as bass
import concourse.tile as tile
from concourse import mybir
from concourse.bass_utils import run_bass_kernel_spmd

F32 = mybir.dt.float32
F32R = mybir.dt.float32r
F16 = mybir.dt.float16

B, S, D, NH, DH = 2, 2048, 1024, 16, 64
HL = 4            # heads per core
DC = D // 128     # 8 d-chunks
NQT = S // 512    # 4 qi tiles
NSC = S // 128    # 16 128-token chunks
IGNORE = -100000.0

# ---------------------------------------------------------------------------
# Workaround for this walrus build's per-instruction sync-wait budget of one
# ("Too many sync wait commands"): after Tile scheduling, move excess waits
# from any instruction onto same-engine NoOps inserted just before it.
MAX_WAITS = 1


def _split_sync_waits(nc, max_waits=MAX_WAITS):
    k = 0
    for fn in nc.m.functions:
        for bb in fn.blocks:
            insts = bb.instructions
            i = 0
            while i < len(insts):
                ins = insts[i]
                si = ins.sync_info
                if si is not None and len(si.on_wait) > max_waits:
                    waits = list(si.on_wait)
                    extra, keep = waits[:-max_waits], waits[-max_waits:]
                    for j in range(0, len(extra), max_waits):
                        nop = mybir.InstNoOp(
                            name=nc.get_next_instruction_name(), ins=[], outs=[])
                        k += 1
                        nop.engine = ins.engine
                        nop.sync_info = mybir.SyncInfo(
                            on_wait=extra[j:j + max_waits], on_update=[])
                        nc.register_instruction(nop, overwrite=True)
                        insts.insert(i, nop)
                        i += 1
                    ins.sync_info = mybir.SyncInfo(
                        on_wait=keep, on_update=list(si.on_update))
                i += 1
    return k


# ---------------------------------------------------------------------------
def _emit(nc, tc, d):
    xT_d, wqk_d, qkb_d, wv_d, vb_d, ostk_d, mask_d, outT_d = d

    with tc.tile_pool(name="persist", bufs=1) as persist:
        xT = persist.tile([128, DC, S], F16)
        wqk = persist.tile([128, DC, 512], F16)
        wv = persist.tile([128, DC, 256], F16)
        qkb = persist.tile([128, 4], F32)
        vb = persist.tile([128, 260], F16)
        ostk = persist.tile([128, 2, DC, 128], F16)
        maskt = persist.tile([128, 2, 896], F16)
        qkT = persist.tile([128, 4, S], F16)
        vaug = persist.tile([128, NSC, HL * 65], F16)
        zstk = persist.tile([128, 2, S], F16)

        # ---- input DMA: sync (HWDGE) + gpsimd (SWDGE) queues only; the
        # scalar queue is kept clear for the exp stream.  Earliest-needed
        # pieces first (st=0 projections can start ~2us in), later x tiles
        # as coarse [128,1536] transfers to amortize the per-dma engine cost.
        wqk_r = wqk_d.rearrange("p (c n) -> p c n", c=DC)
        wv_r = wv_d.rearrange("p (c n) -> p c n", c=DC)
        # critical stream on THREE queues (the scalar/Act queue is idle until
        # the first exp ~15us in, so it can carry early input safely); xT
        # before wqk per dc so the first proj matmul unblocks earliest.
        # weighted rotation: the two HWDGE queues (sync/scalar, ~1us per
        # 128KB chunk) each take 2 of every 5 transfers, the slower SWDGE
        # (gpsimd) takes 1, so the critical stream completes balanced.
        q5 = [nc.sync, nc.scalar, nc.sync, nc.scalar, nc.gpsimd]
        nc.gpsimd.dma_start(out=qkb[:, :], in_=qkb_d[:, :])
        rr = 0
        for dc in range(DC):
            q5[rr % 5].dma_start(out=xT[:, dc, 0:512],
                                 in_=xT_d[dc * 128:(dc + 1) * 128, 0:512]); rr += 1
            q5[rr % 5].dma_start(out=wqk[:, dc, :], in_=wqk_r[:, dc, :]); rr += 1
        nc.sync.dma_start(out=maskt[:, :, :], in_=mask_d.rearrange("p (a n) -> p a n", a=2))
        nc.scalar.dma_start(out=wv[:, 0:4, :], in_=wv_r[:, 0:4, :])
        nc.gpsimd.dma_start(out=wv[:, 4:8, :], in_=wv_r[:, 4:8, :])
        nc.gpsimd.dma_start(out=vb[:, :], in_=vb_d[:, :])
        for dc in range(DC):  # st1 fine-grained (needed by qt0's fillers)
            q5[rr % 5].dma_start(out=xT[:, dc, 512:1024],
                                 in_=xT_d[dc * 128:(dc + 1) * 128, 512:1024]); rr += 1
        qs = [nc.sync, nc.gpsimd]
        for dc in range(DC):
            qs[dc % 2].dma_start(out=xT[:, dc, 1024:S],
                                 in_=xT_d[dc * 128:(dc + 1) * 128, 1024:S])
        nc.sync.dma_start(out=ostk[:, :, :, :], in_=ostk_d.rearrange("p (a d c) -> p a d c", a=2, d=DC))

        ones16 = persist.tile([128, 64], F16)
        nc.vector.memset(ones16[:, :], 1.0)

        with (
            tc.tile_pool(name="psP", bufs=2, space="PSUM") as psP,
            tc.tile_pool(name="psS", bufs=2, space="PSUM") as psS,
            tc.tile_pool(name="psZ", bufs=2, space="PSUM") as psZ,
            tc.tile_pool(name="att", bufs=8) as attp,
            tc.tile_pool(name="nrm", bufs=3) as nrm,
            tc.tile_pool(name="ost", bufs=2) as ostp,
        ):
            og_tiles = {}

            def emit_proj_group(st, g):
                """g 0-3: Q/K r-tiles; g 4-7: V 128-chunks of s-tile st."""
                stw = slice(st * 512, (st + 1) * 512)
                if g < 4:
                    r = g
                    ps = psP.tile([128, 512], F32, tag="proj", name=f"qk_{st}_{r}")
                    for dc in range(DC):
                        nc.tensor.matmul(
                            ps,
                            lhsT=wqk[:, dc, r * 128:(r + 1) * 128],
                            rhs=xT[:, dc, stw],
                            start=(dc == 0), stop=(dc == DC - 1),
                        )
                    nc.vector.tensor_scalar_add(
                        out=qkT[:, r, stw], in0=ps, scalar1=qkb[:, r:r + 1])
                else:
                    sc = 4 * st + (g - 4)
                    ps = psP.tile([128, 256], F32, tag="proj", name=f"v_{sc}")
                    for dc in range(DC):
                        nc.tensor.matmul(
                            ps,
                            lhsT=xT[:, dc, sc * 128:(sc + 1) * 128],
                            rhs=wv[:, dc, :],
                            start=(dc == 0), stop=(dc == DC - 1),
                        )
                    vsl = vaug[:, sc, :].rearrange("p (h c) -> p h c", c=65)
                    nc.vector.tensor_copy(vsl[:, :, 0:64],
                                          ps.rearrange("p (h c) -> p h c", c=64))
                    # st0 runs while the gpsimd queue is still issuing input
                    # DMAs -> keep its vaug ops off gpsimd
                    eng = nc.vector if st == 0 else nc.gpsimd
                    eng.memset(vsl[:, :, 64:65], 1.0)
                    eng.tensor_add(out=vaug[:, sc, :], in0=vaug[:, sc, :], in1=vb)

            def attention(qt, fill, pre_av=None, mid=None):
                stw = slice(qt * 512, (qt + 1) * 512)
                nkc = 4 * (qt + 1)

                def hp_attn(hp):
                    qrt, rt = hp, 2 + hp
                    # head pair's z+denom in one tile: [65, hi, 512]
                    zt = psZ.tile([65, 2, 512], F32, tag="zaug", bufs=1,
                                  name=f"z_{qt}_{hp}")
                    ats = {}

                    def scores(kc):
                        j = kc - 4 * qt  # >=0 on diagonal chunks
                        lo = 128 * j if 0 <= j < 4 else 0
                        sc2 = psS.tile([128, 2, 512], F32, tag="sc",
                                       name=f"sc_{qt}_{hp}_{kc}")
                        for half, p0 in ((0, 0), (1, 64)):
                            nc.tensor.matmul(
                                sc2[:, half, lo:512],
                                lhsT=qkT[p0:p0 + 64, rt, kc * 128:(kc + 1) * 128],
                                rhs=qkT[p0:p0 + 64, qrt, qt * 512 + lo:(qt + 1) * 512],
                                start=True, stop=True,
                                tile_position=(p0, 0),
                            )
                        at = attp.tile([128, 2, 512], F16, tag="at")
                        nc.scalar.activation(out=at[:, :, lo:512], in_=sc2[:, :, lo:512],
                                             func=mybir.ActivationFunctionType.Exp)
                        if 0 <= j < 4:  # causal triangle on the 128-block
                            meng = nc.vector if qt == 0 else nc.gpsimd
                            meng.tensor_mul(
                                out=at[:, :, lo:lo + 128],
                                in0=at[:, :, lo:lo + 128],
                                in1=maskt[:, :, 384:512],
                            )
                        ats[kc] = (at, lo)

                    def av(kc):
                        at, lo = ats.pop(kc)
                        for hi in range(2):
                            nc.tensor.matmul(
                                zt[:, hi, lo:512],
                                lhsT=vaug[:, kc, 65 * (2 * hp + hi):65 * (2 * hp + hi) + 65],
                                rhs=at[:, hi, lo:512],
                                start=(kc == 0), stop=(kc == nkc - 1),
                            )

                    scores(0)
                    for kc in range(1, nkc):
                        scores(kc)
                        fill()
                        if pre_av is not None:
                            pre_av(hp, kc - 1)
                        av(kc - 1)
                    if pre_av is not None:
                        pre_av(hp, nkc - 1)
                    av(nkc - 1)

                    # ---- normalize: z * (1/denom). 1/denom = exp(-ln(denom))
                    # on the scalar engine (one instr per head PAIR); denom
                    # row broadcast across partitions via K=1 f16 matmuls.
                    # The kernel-final head pair instead runs a column-halved
                    # variant (returned closure) so the a-half output
                    # projection overlaps the b-half Ln/Exp/dma chain.
                    tail = (qt == NQT - 1 and hp == 1)
                    if not tail:
                        rd = nrm.tile([65, 2, 512], F32, tag="rd")
                        nc.scalar.activation(out=rd[64:65, :, :], in_=zt[64:65, :, :],
                                             func=mybir.ActivationFunctionType.Ln)
                        rd16 = nrm.tile([65, 2, 512], F16, tag="rd16")
                        nc.scalar.activation(out=rd16[64:65, :, :], in_=rd[64:65, :, :],
                                             func=mybir.ActivationFunctionType.Exp,
                                             scale=-1.0)
                        fill()
                        for hi in range(2):
                            rb = psS.tile([64, 512], F32, tag="sc",
                                          name=f"rb_{qt}_{hp}_{hi}")
                            nc.tensor.matmul(rb, lhsT=ones16[64:65, :],
                                             rhs=rd16[64:65, hi, :], start=True, stop=True)
                            rdb = nrm.tile([64, 512], F32, tag="rdb")
                            nc.vector.tensor_copy(rdb[:, :], rb)
                            if hi == 0:
                                nc.vector.tensor_mul(out=zstk[0:64, hp, stw],
                                                     in0=zt[0:64, 0, :], in1=rdb[:, :])
                            else:
                                zs = nrm.tile([64, 512], F16, tag="zs")
                                nc.vector.tensor_mul(out=zs[:, :], in0=zt[0:64, 1, :],
                                                     in1=rdb[:, :])
                                nc.sync.dma_start(out=zstk[64:128, hp, stw],
                                                  in_=zs[:, :])
                        return None

                    def finalize(halves=2):
                        w = 512 // halves
                        for h in range(halves):
                            cw = slice(h * w, (h + 1) * w)
                            gw = slice(qt * 512 + h * w, qt * 512 + (h + 1) * w)
                            zsu = nrm.tile([65, 2, 512], F32, tag="zsu",
                                           name=f"zsu_{qt}_{hp}_{h}")
                            nc.vector.tensor_copy(zsu[:, :, cw], zt[:, :, cw])
                            rd = nrm.tile([65, 2, 512], F32, tag="rd")
                            nc.scalar.activation(out=rd[64:65, :, cw],
                                                 in_=zsu[64:65, :, cw],
                                                 func=mybir.ActivationFunctionType.Ln)
                            rd16 = nrm.tile([65, 2, 512], F16, tag="rd16")
                            nc.scalar.activation(out=rd16[64:65, :, cw],
                                                 in_=rd[64:65, :, cw],
                                                 func=mybir.ActivationFunctionType.Exp,
                                                 scale=-1.0)
                            # hi=1 first: its zstk write needs an SBUF->SBUF
                            # dma (partition shift), the longest tail chain.
                            for hi in (1, 0):
                                rb = psS.tile([64, 512], F32, tag="sc",
                                              name=f"rb_{qt}_{hp}_{hi}_{h}")
                                nc.tensor.matmul(rb[:, cw], lhsT=ones16[64:65, :],
                                                 rhs=rd16[64:65, hi, cw],
                                                 start=True, stop=True)
                                if hi == 0:
                                    nc.vector.tensor_mul(out=zstk[0:64, hp, gw],
                                                         in0=zsu[0:64, 0, cw],
                                                         in1=rb[:, cw])
                                else:
                                    zs = nrm.tile([64, 512], F16, tag="zs")
                                    nc.vector.tensor_mul(out=zs[:, cw],
                                                         in0=zsu[0:64, 1, cw],
                                                         in1=rb[:, cw])
                                    nc.sync.dma_start(out=zstk[64:128, hp, gw],
                                                      in_=zs[:, cw])

                    return finalize

                hp_attn(0)
                if mid is not None:
                    mid()
                return hp_attn(1)

            def emit_outproj(qt, dc, h=0, halves=1):
                w = 512 // halves
                cw = slice(h * w, (h + 1) * w)
                gw = slice(qt * 512 + h * w, qt * 512 + (h + 1) * w)
                if qt not in og_tiles:
                    og_tiles[qt] = ostp.tile([128, DC, 512], F16, tag="og",
                                             name=f"og_{qt}")
                og = og_tiles[qt]
                po = psP.tile([128, 512], F32, tag="proj", name=f"po_{qt}_{dc}_{h}")
                for pair in range(2):
                    nc.tensor.matmul(
                        po[:, cw],
                        lhsT=ostk[:, pair, dc, :],
                        rhs=zstk[:, pair, gw],
                        start=(pair == 0), stop=(pair == 1),
                    )
                # psum -> staged f16 output tile; on the last s-tile split the
                # copies across vector+scalar (the exp stream is over by then)
                if qt == NQT - 1 and dc % 2 == 1:
                    nc.scalar.activation(out=og[:, dc, cw], in_=po[:, cw],
                                         func=mybir.ActivationFunctionType.Copy)
                else:
                    nc.vector.tensor_copy(og[:, dc, cw], po[:, cw])
                if dc == DC - 1:
                    outT_r = outT_d.rearrange("(d p) s -> p d s", p=128)
                    # last half of the last s-tile goes on gpsimd so the two
                    # final output transfers drain on independent queues
                    q = nc.gpsimd if (qt == NQT - 1 and h == halves - 1) else nc.sync
                    q.dma_start(out=outT_r[:, :, gw], in_=og[:, :, cw])

            # bootstrap: only the two hp0 Q/K groups up front; the rest of
            # proj(0) is emitted just-in-time inside attention(0) (V chunk
            # right before its first attn@V use; hp1's Q/K between the hp
            # halves) so the scalar exp stream starts as soon as possible.
            emit_proj_group(0, 0)
            emit_proj_group(0, 2)
            emitted_v0 = set()

            def pre_av0(hp, kc):
                if hp == 0 and kc not in emitted_v0:
                    emitted_v0.add(kc)
                    emit_proj_group(0, 4 + kc)

            def mid0():
                emit_proj_group(0, 1)
                emit_proj_group(0, 3)

            # fillers per qt window (see module docstring):
            #   qt0: proj(st1)            qt1: proj(st2)
            #   qt2: proj(st3)+outproj(0) qt3: outproj(1)+outproj(2)
            filler_map = {
                0: [lambda g=g: emit_proj_group(1, g) for g in range(8)],
                1: [lambda g=g: emit_proj_group(2, g) for g in range(8)],
                2: ([lambda g=g: emit_proj_group(3, g) for g in range(8)]
                    + [lambda d=d: emit_outproj(0, d) for d in range(DC)]),
                3: ([lambda d=d: emit_outproj(1, d) for d in range(DC)]
                    + [lambda d=d: emit_outproj(2, d) for d in range(DC)]),
            }

            prev_fin = None
            for st in range(NQT):
                fillers = filler_map[st]
                nslots = 2 * 4 * (st + 1)  # fill() calls in attention(st)
                # fire filler i at slot floor(i*nslots/n) -> even spread
                n = len(fillers)
                # qt0: hold fillers back two slots so the scheduler can't
                # hoist a dma-gated proj(st1) matmul ahead of the first scores
                off = 2 if st == 0 else 0
                sched = [off + (i * (nslots - off)) // n for i in range(n)]
                state = {"slot": 0, "idx": 0}

                def fill(fillers=fillers, sched=sched, state=state):
                    s = state["slot"]
                    while state["idx"] < len(fillers) and sched[state["idx"]] <= s:
                        fillers[state["idx"]]()
                        state["idx"] += 1
                    state["slot"] += 1

                prev_fin = attention(st, fill,
                                     pre_av=pre_av0 if st == 0 else None,
                                     mid=mid0 if st == 0 else None)
                while state["idx"] < len(fillers):
                    fillers[state["idx"]]()
                    state["idx"] += 1
            # tail: finalize the last head pair in column halves; the a-half
            # output projection overlaps the b-half Ln/Exp/dma chain.
            prev_fin(halves=2)
            for h in range(2):
                for dc in range(DC):
                    emit_outproj(NQT - 1, dc, h=h, halves=2)


def build_nc() -> bass.Bass:
    nc = bass.Bass()
    xT_d = nc.dram_tensor("xT", [D, S], F16, kind="ExternalInput")
    wqk_d = nc.dram_tensor("wqk", [128, DC * 512], F16, kind="ExternalInput")
    qkb_d = nc.dram_tensor("qkb", [128, 4], F32, kind="ExternalInput")
    wv_d = nc.dram_tensor("wv", [128, DC * 256], F16, kind="ExternalInput")
    vb_d = nc.dram_tensor("vb", [128, 260], F16, kind="ExternalInput")
    ostk_d = nc.dram_tensor("ostk", [128, 2 * DC * 128], F16, kind="ExternalInput")
    mask_d = nc.dram_tensor("mask", [128, 2 * 896], F16, kind="ExternalInput")
    outT_d = nc.dram_tensor("outT", [D, S], F16, kind="ExternalOutput")

    with tile.TileContext(nc) as tc:
        _emit(nc, tc, (xT_d, wqk_d, qkb_d, wv_d, vb_d, ostk_d, mask_d, outT_d))
    _split_sync_waits(nc)
    return nc


# ---------------------------------------------------------------------------
def _prep_core_inputs(c, x, Qs, Qbs, Ks, Kbs, Vs, Vbs, O):
    b, hg = divmod(c, 4)
    heads = list(range(4 * hg, 4 * hg + 4))
    scale = np.float32(1.0 / np.sqrt(DH))

    xT = np.ascontiguousarray(x[b].T, dtype=np.float16)

    wq = np.concatenate([Qs[h] for h in heads], axis=1)
    wk = np.concatenate([Ks[h] for h in heads], axis=1) * scale
    wqk = np.concatenate([wq, wk], axis=1).astype(np.float16)
    wqk = np.ascontiguousarray(
        wqk.reshape(DC, 128, 512).transpose(1, 0, 2).reshape(128, DC * 512))

    qkb_cols = np.concatenate([Qbs[h] for h in heads] + [Kbs[h] * scale for h in heads])
    qkb = np.ascontiguousarray(qkb_cols.reshape(4, 128).T, dtype=np.float32)

    wv = np.concatenate([Vs[h] for h in heads], axis=1).astype(np.float16)
    wv = np.ascontiguousarray(
        wv.reshape(DC, 128, 256).transpose(1, 0, 2).reshape(128, DC * 256))
    vb = np.zeros((128, 260), dtype=np.float16)
    for hh, h in enumerate(heads):
        vb[:, 65 * hh: 65 * hh + 64] = Vbs[h][None, :]

    o4 = np.stack([O[h] for h in heads])                # [4, 64, 1024]
    ostk = o4.reshape(2, 128, DC, 128).transpose(0, 2, 1, 3).astype(np.float16)
    # [pair, dc, r, c] -> partition-major [r, pair, dc, c] flattened
    ostk = np.ascontiguousarray(
        ostk.transpose(2, 0, 1, 3).reshape(128, 2 * DC * 128))

    t = np.arange(896, dtype=np.int64)[None, :] - 384
    i = np.arange(128, dtype=np.int64)[:, None]
    mask = np.where(t >= i, np.float16(1.0), np.float16(0.0)).astype(np.float16)
    mask2 = np.concatenate([mask, mask], axis=1)

    return {"xT": xT, "wqk": wqk, "qkb": qkb, "wv": wv, "vb": vb,
            "ostk": ostk, "mask": np.ascontiguousarray(mask2)}


def _run(inputs, trace=False, tmpdir=None):
    x = np.asarray(inputs["normalized_resid_pre"], dtype=np.float32)
    Qs = np.asarray(inputs["Qs"], dtype=np.float32)
    Qbs = np.asarray(inputs["Qbs"], dtype=np.float32)
    Ks = np.asarray(inputs["Ks"], dtype=np.float32)
    Kbs = np.asarray(inputs["Kbs"], dtype=np.float32)
    Vs = np.asarray(inputs["Vs"], dtype=np.float32)
    Vbs = np.asarray(inputs["Vbs"], dtype=np.float32)
    O = np.asarray(inputs["O"], dtype=np.float32)
    Ob = np.asarray(inputs["Ob"], dtype=np.float32)

    in_maps = [_prep_core_inputs(c, x, Qs, Qbs, Ks, Kbs, Vs, Vbs, O)
               for c in range(8)]
    last_err = None
    for attempt in range(3):
        try:
            nc = build_nc()
            res = run_bass_kernel_spmd(nc, in_maps, list(range(8)), trace=trace,
                                       tmpdir=tmpdir)
            break
        except Exception as e:  # transient NRT device errors; retry
            last_err = e
    else:
        raise last_err

    out = np.zeros((B, S, D), dtype=np.float32)
    for c in range(8):
        out[c // 4] += res.results[c]["outT"].T.astype(np.float32)
    out += Ob[None, None, :]
    return out, res


def kernel(**inputs) -> np.ndarray:
    out, _ = _run(inputs, trace=False)
    return out


# revision 33
# speedup vs baseline: 1.0166x; 1.0166x over previous
"""Causal multi-head decoder attention on 8 Trainium2 NeuronCores.

Problem shapes (hardcoded): x [B=2, S=2048, D=1024], 16 heads x d_head=64.
Sharding: core c -> (batch b = c//4, head-group hg = c%4 covering 4 heads).
Attention is fully head-local; each core computes the partial output
projection for its 4 heads, and the host sums the 4 partials per batch
(the "output projection all-reduce") during unshard.

On-device layout strategy (per core):
  - host provides xT = x[b].T  [1024, 2048] so Q/K projections directly
    produce qT/kT [64, S] (head-dim on partitions) with no transposes.
  - Q/K projections produce head PAIRS stacked in partition halves
    (qkT[:, r, :]: rows 0:64 = head 2r', 64:128 = head 2r'+1), so the
    score matmuls for a head pair run as two concurrent 64-row-group
    matmuls on the PE array (tile_position) with NO duplication DMAs.
  - V is computed in [S, 64] orientation (x-chunk stationary) and stored
    interleaved with a ones-column per head: vaug [128, 16sc, 4h*65].
    The ones-column makes the attn@V matmul also produce the softmax
    denominator row (zaug [65, 512] = 64 z rows + 1 denom row).
  - scoresT [ki, qi] = kT-chunk.T @ qT-tile (contraction over d_head=64).
    exp() on the scalar engine; causal mask applied multiplicatively on
    the diagonal 128-blocks only; for diagonal chunks the score matmul /
    exp / attn@V are all narrowed to the un-masked qi column range.
  - kc ascending accumulation for zaug; the kc loop is software-pipelined
    depth 2 (scores(kc+1) issued before attn@V(kc)).
  - 1/sqrt(d_head) folded into the K weights host-side.
  - softmax 1/denom via DVE reciprocal_approx_fast on the zaug denom row
    (f32, no scalar-engine work), broadcast across partitions with a
    K=1 f32r matmul against a ones row (f32r streams at full rate for
    free dims >= 256, so no f16 conversion pass is needed).
  - output projection: O stacked per head-pair so contraction is 128-wide.
    Output tiles are staged into an SBUF buffer per s-tile and shipped
    with ONE dma per s-tile on the sync queue.
  - engine load-balancing across the qt loop: the scalar-engine exp load
    grows linearly with qt (causal) while tensor work shrinks, so output
    projections are deferred into late qt windows as PE filler work:
      qt2 <- outproj(st0), qt3 <- outproj(st1) + outproj(st2),
      tail <- outproj(st3); fillers are spread EVENLY over the kc loop.
  - input DMA on sync+gpsimd queues only (scalar queue stays clear for
    the exp stream), first s-tile chunks first.
"""

import os as _os

import numpy as np

import concourse.bass as bass
import concourse.tile as tile
from concourse import mybir
from concourse.bass_utils import run_bass_kernel_spmd

F32 = mybir.dt.float32
F32R = mybir.dt.float32r
F16 = mybir.dt.float16

B, S, D, NH, DH = 2, 2048, 1024, 16, 64
HL = 4            # heads per core
DC = D // 128     # 8 d-chunks
NQT = S // 512    # 4 qi tiles
NSC = S // 128    # 16 128-token chunks
IGNORE = -100000.0

# ---------------------------------------------------------------------------
# Workaround for this walrus build's per-instruction sync-wait budget of one
# ("Too many sync wait commands"): after Tile scheduling, move excess waits
# from any instruction onto same-engine NoOps inserted just before it.
MAX_WAITS = 1


def _split_sync_waits(nc, max_waits=MAX_WAITS):
    k = 0
    for fn in nc.m.functions:
        for bb in fn.blocks:
            insts = bb.instructions
            i = 0
            while i < len(insts):
                ins = insts[i]
                si = ins.sync_info
                if si is not None and len(si.on_wait) > max_waits:
                    waits = list(si.on_wait)
                    extra, keep = waits[:-max_waits], waits[-max_waits:]
                    for j in range(0, len(extra), max_waits):
                        nop = mybir.InstNoOp(
                            name=nc.get_next_instruction_name(), ins=[], outs=[])
                        k += 1
                        nop.engine = ins.engine
                        nop.sync_info = mybir.SyncInfo(
                            on_wait=extra[j:j + max_waits], on_update=[])
                        nc.register_instruction(nop, overwrite=True)
                        insts.insert(i, nop)
                        i += 1
                    ins.sync_info = mybir.SyncInfo(
                        on_wait=keep, on_update=list(si.on_update))
                i += 1
    return k


# ---------------------------------------------------------------------------
def _emit(nc, tc, d):
    xT_d, wqk_d, qkb_d, wv_d, vb_d, ostk_d, mask_d, outT_d = d

    with tc.tile_pool(name="persist", bufs=1) as persist:
        xT = persist.tile([128, DC, S], F16)
        wqk = persist.tile([128, DC, 512], F16)
        wv = persist.tile([128, DC, 256], F16)
        qkb = persist.tile([128, 4], F32)
        vb = persist.tile([128, 260], F16)
        ostk = persist.tile([128, 2, DC, 128], F16)
        maskt = persist.tile([128, 2, 896], F16)
        qkT = persist.tile([128, 4, S], F16)
        vaug = persist.tile([128, NSC, HL * 65], F16)
        zstk = persist.tile([128, 2, S], F16)

        # ---- input DMA: sync (HWDGE) + gpsimd (SWDGE) queues only; the
        # scalar queue is kept clear for the exp stream.  Earliest-needed
        # pieces first (st=0 projections can start ~2us in), later x tiles
        # as coarse [128,1536] transfers to amortize the per-dma engine cost.
        wqk_r = wqk_d.rearrange("p (c n) -> p c n", c=DC)
        wv_r = wv_d.rearrange("p (c n) -> p c n", c=DC)
        # critical stream on THREE queues (the scalar/Act queue is idle until
        # the first exp ~15us in, so it can carry early input safely); xT
        # before wqk per dc so the first proj matmul unblocks earliest.
        # Each dma costs its queue ~0.65-1us mostly INDEPENDENT of size
        # (completion-receipt dominated), so the input streams use coarse
        # 256-512KB transfers, spread over all three dma-capable queues
        # (the scalar/Act queue is idle until the first exp ~15us in).
        xT_r = xT_d.rearrange("(c p) s -> p c s", p=128)
        q3 = [nc.sync, nc.scalar, nc.gpsimd]
        nc.gpsimd.dma_start(out=qkb[:, :], in_=qkb_d[:, :])
        rr = 0
        for dc in range(0, DC, 2):  # criticals: xT st0 + wqk, dc-pair chunks
            q3[rr % 3].dma_start(out=xT[:, dc:dc + 2, 0:512],
                                 in_=xT_r[:, dc:dc + 2, 0:512]); rr += 1
            q3[rr % 3].dma_start(out=wqk[:, dc:dc + 2, :], in_=wqk_r[:, dc:dc + 2, :]); rr += 1
        nc.gpsimd.dma_start(out=maskt[:, :, :], in_=mask_d.rearrange("p (a n) -> p a n", a=2))
        nc.sync.dma_start(out=wv[:, 0:4, :], in_=wv_r[:, 0:4, :])
        nc.scalar.dma_start(out=wv[:, 4:8, :], in_=wv_r[:, 4:8, :])
        nc.gpsimd.dma_start(out=vb[:, :], in_=vb_d[:, :])
        for dc in range(0, DC, 2):  # st1 (needed by qt0's fillers)
            q3[rr % 3].dma_start(out=xT[:, dc:dc + 2, 512:1024],
                                 in_=xT_r[:, dc:dc + 2, 512:1024]); rr += 1
        for dc in range(0, DC, 2):  # st2+st3
            q3[rr % 3].dma_start(out=xT[:, dc:dc + 2, 1024:S],
                                 in_=xT_r[:, dc:dc + 2, 1024:S]); rr += 1
        nc.sync.dma_start(out=ostk[:, :, :, :], in_=ostk_d.rearrange("p (a d c) -> p a d c", a=2, d=DC))

        ones16 = persist.tile([128, 64], F16)
        nc.vector.memset(ones16[:, :], 1.0)

        with (
            tc.tile_pool(name="psP", bufs=2, space="PSUM") as psP,
            tc.tile_pool(name="psS", bufs=2, space="PSUM") as psS,
            tc.tile_pool(name="psZ", bufs=2, space="PSUM") as psZ,
            tc.tile_pool(name="att", bufs=8) as attp,
            tc.tile_pool(name="nrm", bufs=3) as nrm,
            tc.tile_pool(name="ost", bufs=2) as ostp,
        ):
            og_tiles = {}

            def emit_proj_group(st, g):
                """g 0-3: Q/K r-tiles; g 4-7: V 128-chunks of s-tile st."""
                stw = slice(st * 512, (st + 1) * 512)
                if g < 4:
                    r = g
                    ps = psP.tile([128, 512], F32, tag="proj", name=f"qk_{st}_{r}")
                    for dc in range(DC):
                        nc.tensor.matmul(
                            ps,
                            lhsT=wqk[:, dc, r * 128:(r + 1) * 128],
                            rhs=xT[:, dc, stw],
                            start=(dc == 0), stop=(dc == DC - 1),
                        )
                    nc.vector.tensor_scalar_add(
                        out=qkT[:, r, stw], in0=ps, scalar1=qkb[:, r:r + 1])
                else:
                    sc = 4 * st + (g - 4)
                    ps = psP.tile([128, 256], F32, tag="proj", name=f"v_{sc}")
                    for dc in range(DC):
                        nc.tensor.matmul(
                            ps,
                            lhsT=xT[:, dc, sc * 128:(sc + 1) * 128],
                            rhs=wv[:, dc, :],
                            start=(dc == 0), stop=(dc == DC - 1),
                        )
                    vsl = vaug[:, sc, :].rearrange("p (h c) -> p h c", c=65)
                    nc.vector.tensor_copy(vsl[:, :, 0:64],
                                          ps.rearrange("p (h c) -> p h c", c=64))
                    # st0 runs while the gpsimd queue is still issuing input
                    # DMAs -> keep its vaug ops off gpsimd
                    eng = nc.vector if st == 0 else nc.gpsimd
                    eng.memset(vsl[:, :, 64:65], 1.0)
                    eng.tensor_add(out=vaug[:, sc, :], in0=vaug[:, sc, :], in1=vb)

            def attention(qt, fill, pre_av=None, mid=None):
                stw = slice(qt * 512, (qt + 1) * 512)
                nkc = 4 * (qt + 1)

                def hp_attn(hp):
                    qrt, rt = hp, 2 + hp
                    # head pair's z+denom in one tile: [65, hi, 512]
                    zt = psZ.tile([65, 2, 512], F32, tag="zaug", bufs=1,
                                  name=f"z_{qt}_{hp}")
                    ats = {}

                    def scores(kc):
                        j = kc - 4 * qt  # >=0 on diagonal chunks
                        lo = 128 * j if 0 <= j < 4 else 0
                        sc2 = psS.tile([128, 2, 512], F32, tag="sc",
                                       name=f"sc_{qt}_{hp}_{kc}")
                        for half, p0 in ((0, 0), (1, 64)):
                            nc.tensor.matmul(
                                sc2[:, half, lo:512],
                                lhsT=qkT[p0:p0 + 64, rt, kc * 128:(kc + 1) * 128],
                                rhs=qkT[p0:p0 + 64, qrt, qt * 512 + lo:(qt + 1) * 512],
                                start=True, stop=True,
                                tile_position=(p0, 0),
                            )
                        at = attp.tile([128, 2, 512], F16, tag="at")
                        nc.scalar.activation(out=at[:, :, lo:512], in_=sc2[:, :, lo:512],
                                             func=mybir.ActivationFunctionType.Exp)
                        if 0 <= j < 4:  # causal triangle on the 128-block
                            meng = nc.vector if qt == 0 else nc.gpsimd
                            meng.tensor_mul(
                                out=at[:, :, lo:lo + 128],
                                in0=at[:, :, lo:lo + 128],
                                in1=maskt[:, :, 384:512],
                            )
                        ats[kc] = (at, lo)

                    def av(kc):
                        at, lo = ats.pop(kc)
                        for hi in range(2):
                            nc.tensor.matmul(
                                zt[:, hi, lo:512],
                                lhsT=vaug[:, kc, 65 * (2 * hp + hi):65 * (2 * hp + hi) + 65],
                                rhs=at[:, hi, lo:512],
                                start=(kc == 0), stop=(kc == nkc - 1),
                            )

                    scores(0)
                    for kc in range(1, nkc):
                        scores(kc)
                        fill()
                        if pre_av is not None:
                            pre_av(hp, kc - 1)
                        av(kc - 1)
                    if pre_av is not None:
                        pre_av(hp, nkc - 1)
                    av(nkc - 1)

                    # ---- normalize: z * (1/denom). 1/denom = exp(-ln(denom))
                    # on the scalar engine (one instr per head PAIR); denom
                    # row broadcast across partitions via K=1 f16 matmuls.
                    # The kernel-final head pair instead runs a column-halved
                    # variant (returned closure) so the a-half output
                    # projection overlaps the b-half Ln/Exp/dma chain.
                    tail = (qt == NQT - 1 and hp == 1)
                    if not tail:
                        rd = nrm.tile([65, 2, 512], F32, tag="rd")
                        nc.scalar.activation(out=rd[64:65, :, :], in_=zt[64:65, :, :],
                                             func=mybir.ActivationFunctionType.Ln)
                        rd16 = nrm.tile([65, 2, 512], F16, tag="rd16")
                        nc.scalar.activation(out=rd16[64:65, :, :], in_=rd[64:65, :, :],
                                             func=mybir.ActivationFunctionType.Exp,
                                             scale=-1.0)
                        fill()
                        for hi in range(2):
                            rb = psS.tile([64, 512], F32, tag="sc",
                                          name=f"rb_{qt}_{hp}_{hi}")
                            nc.tensor.matmul(rb, lhsT=ones16[64:65, :],
                                             rhs=rd16[64:65, hi, :], start=True, stop=True)
                            rdb = nrm.tile([64, 512], F32, tag="rdb")
                            nc.vector.tensor_copy(rdb[:, :], rb)
                            if hi == 0:
                                nc.vector.tensor_mul(out=zstk[0:64, hp, stw],
                                                     in0=zt[0:64, 0, :], in1=rdb[:, :])
                            else:
                                zs = nrm.tile([64, 512], F16, tag="zs")
                                nc.vector.tensor_mul(out=zs[:, :], in0=zt[0:64, 1, :],
                                                     in1=rdb[:, :])
                                nc.sync.dma_start(out=zstk[64:128, hp, stw],
                                                  in_=zs[:, :])
                        return None

                    def finalize(halves=2):
                        w = 512 // halves
                        for h in range(halves):
                            cw = slice(h * w, (h + 1) * w)
                            gw = slice(qt * 512 + h * w, qt * 512 + (h + 1) * w)
                            zsu = nrm.tile([65, 2, 512], F32, tag="zsu",
                                           name=f"zsu_{qt}_{hp}_{h}")
                            nc.vector.tensor_copy(zsu[:, :, cw], zt[:, :, cw])
                            rd = nrm.tile([65, 2, 512], F32, tag="rd")
                            nc.scalar.activation(out=rd[64:65, :, cw],
                                                 in_=zsu[64:65, :, cw],
                                                 func=mybir.ActivationFunctionType.Ln)
                            rd16 = nrm.tile([65, 2, 512], F16, tag="rd16")
                            nc.scalar.activation(out=rd16[64:65, :, cw],
                                                 in_=rd[64:65, :, cw],
                                                 func=mybir.ActivationFunctionType.Exp,
                                                 scale=-1.0)
                            # hi=1 first: its zstk write needs an SBUF->SBUF
                            # dma (partition shift), the longest tail chain.
                            for hi in (1, 0):
                                rb = psS.tile([64, 512], F32, tag="sc",
                                              name=f"rb_{qt}_{hp}_{hi}_{h}")
                                nc.tensor.matmul(rb[:, cw], lhsT=ones16[64:65, :],
                                                 rhs=rd16[64:65, hi, cw],
                                                 start=True, stop=True)
                                if hi == 0:
                                    nc.vector.tensor_mul(out=zstk[0:64, hp, gw],
                                                         in0=zsu[0:64, 0, cw],
                                                         in1=rb[:, cw])
                                else:
                                    zs = nrm.tile([64, 512], F16, tag="zs")
                                    nc.vector.tensor_mul(out=zs[:, cw],
                                                         in0=zsu[0:64, 1, cw],
                                                         in1=rb[:, cw])
                                    nc.sync.dma_start(out=zstk[64:128, hp, gw],
                                                      in_=zs[:, cw])

                    return finalize

                hp_attn(0)
                if mid is not None:
                    mid()
                return hp_attn(1)

            def emit_outproj(qt, dc, h=0, halves=1):
                w = 512 // halves
                cw = slice(h * w, (h + 1) * w)
                gw = slice(qt * 512 + h * w, qt * 512 + (h + 1) * w)
                if qt not in og_tiles:
                    og_tiles[qt] = ostp.tile([128, DC, 512], F16, tag="og",
                                             name=f"og_{qt}")
                og = og_tiles[qt]
                po = psP.tile([128, 512], F32, tag="proj", name=f"po_{qt}_{dc}_{h}")
                for pair in range(2):
                    nc.tensor.matmul(
                        po[:, cw],
                        lhsT=ostk[:, pair, dc, :],
                        rhs=zstk[:, pair, gw],
                        start=(pair == 0), stop=(pair == 1),
                    )
                # psum -> staged f16 output tile; on the last s-tile split the
                # copies across vector+scalar (the exp stream is over by then)
                if qt == NQT - 1 and dc % 2 == 1:
                    nc.scalar.activation(out=og[:, dc, cw], in_=po[:, cw],
                                         func=mybir.ActivationFunctionType.Copy)
                else:
                    nc.vector.tensor_copy(og[:, dc, cw], po[:, cw])
                if dc == DC - 1:
                    outT_r = outT_d.rearrange("(d p) s -> p d s", p=128)
                    # last half of the last s-tile goes on gpsimd so the two
                    # final output transfers drain on independent queues
                    q = nc.gpsimd if (qt == NQT - 1 and h == halves - 1) else nc.sync
                    q.dma_start(out=outT_r[:, :, gw], in_=og[:, :, cw])

            # bootstrap: only the two hp0 Q/K groups up front; the rest of
            # proj(0) is emitted just-in-time inside attention(0) (V chunk
            # right before its first attn@V use; hp1's Q/K between the hp
            # halves) so the scalar exp stream starts as soon as possible.
            emit_proj_group(0, 0)
            emit_proj_group(0, 2)
            emitted_v0 = set()

            def pre_av0(hp, kc):
                if hp == 0 and kc not in emitted_v0:
                    emitted_v0.add(kc)
                    emit_proj_group(0, 4 + kc)

            def mid0():
                emit_proj_group(0, 1)
                emit_proj_group(0, 3)

            # fillers per qt window (see module docstring):
            #   qt0: proj(st1)            qt1: proj(st2)
            #   qt2: proj(st3)+outproj(0) qt3: outproj(1)+outproj(2)
            filler_map = {
                0: [lambda g=g: emit_proj_group(1, g) for g in range(8)],
                1: [lambda g=g: emit_proj_group(2, g) for g in range(8)],
                2: ([lambda g=g: emit_proj_group(3, g) for g in range(8)]
                    + [lambda d=d: emit_outproj(0, d) for d in range(DC)]),
                3: ([lambda d=d: emit_outproj(1, d) for d in range(DC)]
                    + [lambda d=d: emit_outproj(2, d) for d in range(DC)]),
            }

            prev_fin = None
            for st in range(NQT):
                fillers = filler_map[st]
                nslots = 2 * 4 * (st + 1)  # fill() calls in attention(st)
                # fire filler i at slot floor(i*nslots/n) -> even spread
                n = len(fillers)
                # qt0: hold fillers back two slots so the scheduler can't
                # hoist a dma-gated proj(st1) matmul ahead of the first scores
                off = 2 if st == 0 else 0
                sched = [off + (i * (nslots - off)) // n for i in range(n)]
                state = {"slot": 0, "idx": 0}

                def fill(fillers=fillers, sched=sched, state=state):
                    s = state["slot"]
                    while state["idx"] < len(fillers) and sched[state["idx"]] <= s:
                        fillers[state["idx"]]()
                        state["idx"] += 1
                    state["slot"] += 1

                prev_fin = attention(st, fill,
                                     pre_av=pre_av0 if st == 0 else None,
                                     mid=mid0 if st == 0 else None)
                while state["idx"] < len(fillers):
                    fillers[state["idx"]]()
                    state["idx"] += 1
            # tail: finalize the last head pair in column halves; the a-half
            # output projection overlaps the b-half Ln/Exp/dma chain.
            prev_fin(halves=2)
            for h in range(2):
                for dc in range(DC):
                    emit_outproj(NQT - 1, dc, h=h, halves=2)


def build_nc() -> bass.Bass:
    nc = bass.Bass()
    xT_d = nc.dram_tensor("xT", [D, S], F16, kind="ExternalInput")
    wqk_d = nc.dram_tensor("wqk", [128, DC * 512], F16, kind="ExternalInput")
    qkb_d = nc.dram_tensor("qkb", [128, 4], F32, kind="ExternalInput")
    wv_d = nc.dram_tensor("wv", [128, DC * 256], F16, kind="ExternalInput")
    vb_d = nc.dram_tensor("vb", [128, 260], F16, kind="ExternalInput")
    ostk_d = nc.dram_tensor("ostk", [128, 2 * DC * 128], F16, kind="ExternalInput")
    mask_d = nc.dram_tensor("mask", [128, 2 * 896], F16, kind="ExternalInput")
    outT_d = nc.dram_tensor("outT", [D, S], F16, kind="ExternalOutput")

    with tile.TileContext(nc) as tc:
        _emit(nc, tc, (xT_d, wqk_d, qkb_d, wv_d, vb_d, ostk_d, mask_d, outT_d))
    _split_sync_waits(nc)
    return nc


# ---------------------------------------------------------------------------
def _prep_core_inputs(c, x, Qs, Qbs, Ks, Kbs, Vs, Vbs, O):
    b, hg = divmod(c, 4)
    heads = list(range(4 * hg, 4 * hg + 4))
    scale = np.float32(1.0 / np.sqrt(DH))

    xT = np.ascontiguousarray(x[b].T, dtype=np.float16)

    wq = np.concatenate([Qs[h] for h in heads], axis=1)
    wk = np.concatenate([Ks[h] for h in heads], axis=1) * scale
    wqk = np.concatenate([wq, wk], axis=1).astype(np.float16)
    wqk = np.ascontiguousarray(
        wqk.reshape(DC, 128, 512).transpose(1, 0, 2).reshape(128, DC * 512))

    qkb_cols = np.concatenate([Qbs[h] for h in heads] + [Kbs[h] * scale for h in heads])
    qkb = np.ascontiguousarray(qkb_cols.reshape(4, 128).T, dtype=np.float32)

    wv = np.concatenate([Vs[h] for h in heads], axis=1).astype(np.float16)
    wv = np.ascontiguousarray(
        wv.reshape(DC, 128, 256).transpose(1, 0, 2).reshape(128, DC * 256))
    vb = np.zeros((128, 260), dtype=np.float16)
    for hh, h in enumerate(heads):
        vb[:, 65 * hh: 65 * hh + 64] = Vbs[h][None, :]

    o4 = np.stack([O[h] for h in heads])                # [4, 64, 1024]
    ostk = o4.reshape(2, 128, DC, 128).transpose(0, 2, 1, 3).astype(np.float16)
    # [pair, dc, r, c] -> partition-major [r, pair, dc, c] flattened
    ostk = np.ascontiguousarray(
        ostk.transpose(2, 0, 1, 3).reshape(128, 2 * DC * 128))

    t = np.arange(896, dtype=np.int64)[None, :] - 384
    i = np.arange(128, dtype=np.int64)[:, None]
    mask = np.where(t >= i, np.float16(1.0), np.float16(0.0)).astype(np.float16)
    mask2 = np.concatenate([mask, mask], axis=1)

    return {"xT": xT, "wqk": wqk, "qkb": qkb, "wv": wv, "vb": vb,
            "ostk": ostk, "mask": np.ascontiguousarray(mask2)}


def _run(inputs, trace=False, tmpdir=None):
    x = np.asarray(inputs["normalized_resid_pre"], dtype=np.float32)
    Qs = np.asarray(inputs["Qs"], dtype=np.float32)
    Qbs = np.asarray(inputs["Qbs"], dtype=np.float32)
    Ks = np.asarray(inputs["Ks"], dtype=np.float32)
    Kbs = np.asarray(inputs["Kbs"], dtype=np.float32)
    Vs = np.asarray(inputs["Vs"], dtype=np.float32)
    Vbs = np.asarray(inputs["Vbs"], dtype=np.float32)
    O = np.asarray(inputs["O"], dtype=np.float32)
    Ob = np.asarray(inputs["Ob"], dtype=np.float32)

    in_maps = [_prep_core_inputs(c, x, Qs, Qbs, Ks, Kbs, Vs, Vbs, O)
               for c in range(8)]
    last_err = None
    for attempt in range(3):
        try:
            nc = build_nc()
            res = run_bass_kernel_spmd(nc, in_maps, list(range(8)), trace=trace,
                                       tmpdir=tmpdir)
            break
        except Exception as e:  # transient NRT device errors; retry
            last_err = e
    else:
        raise last_err

    out = np.zeros((B, S, D), dtype=np.float32)
    for c in range(8):
        out[c // 4] += res.results[c]["outT"].T.astype(np.float32)
    out += Ob[None, None, :]
    return out, res


def kernel(**inputs) -> np.ndarray:
    out, _ = _run(inputs, trace=False)
    return out
